# revision 6
# baseline (speedup 1.0000x reference)
"""GAT layer kernel for Trainium2 (8 NeuronCores, data-parallel over batch).

Reference computation (per graph b):
    Wh  = atoms @ W                      (N, FO)
    s1  = Wh @ a1 ; s2 = Wh @ a2         (N,)
    e   = leaky_relu(s1[:,None]+s2[None,:], 0.1)
    att = softmax(where(adj>0, e, -9e15), axis=1)
    out = elu(att @ Wh)

Key algebraic identity used on-device: with s = s1_i + s2_j and any fixed
shift M >= max(s),
    exp(leaky_relu(s) - M) = max( e^{s1_i-M/2} e^{s2_j-M/2},
                                  e^{0.1 s1_i-M/2} e^{0.1 s2_j-M/2} )
i.e. a max of two rank-1 outer products -- no transcendental is ever
evaluated on the NxN score matrix.  The 0/1 adjacency multiplies the result
(post-exp masking is exact: masked entries contribute 0 to numerator and
denominator, same as the reference's -9e15 trick).  The softmax denominator
comes for free as a ones-column appended to Wh in the P^T @ [Wh|1] matmul.

Layouts: score tiles are [i partitions, j free] (adjacency loads at line
rate); P blocks are transposed on the PE (128x128) so the attention matmul
contracts j on partitions.
"""

import numpy as np
from contextlib import ExitStack

import concourse.bass as bass
import concourse.tile as tile
import concourse.mybir as mybir
from concourse.masks import make_identity

dt = mybir.dt
Alu = mybir.AluOpType
Act = mybir.ActivationFunctionType

N = 1024          # nodes per graph
F_IN = 128        # input features
FO = 64           # output features
P = 128           # partitions
NCH = N // P      # 8 node chunks
N_CORES = 8
B_FULL = 64
M_SHIFT = 40.0    # fixed softmax shift; any value >= max(s1_i+s2_j) works


def build_gat(bpc: int) -> bass.Bass:
    """Emit the bass program for one core processing `bpc` graphs."""
    nc = bass.Bass()
    atoms = nc.declare_dram_parameter("atoms", [bpc, N, F_IN], dt.float32, isOutput=False)
    adj = nc.declare_dram_parameter("adj", [bpc, N, N], dt.int32, isOutput=False)
    wext = nc.declare_dram_parameter("wext", [F_IN, FO + 1], dt.float32, isOutput=False)
    wa2 = nc.declare_dram_parameter("wa2", [F_IN, 1], dt.float32, isOutput=False)
    out = nc.declare_dram_parameter("out", [bpc, N, FO], dt.float32, isOutput=True)

    with tile.TileContext(nc) as tc, ExitStack() as ctx:
        consts = ctx.enter_context(tc.tile_pool(name="consts", bufs=1))
        psum = ctx.enter_context(tc.tile_pool(name="psum", bufs=6, space="PSUM"))
        gbuf = ctx.enter_context(tc.tile_pool(name="gbuf", bufs=2))
        cbuf = ctx.enter_context(tc.tile_pool(name="cbuf", bufs=2))
        adjbuf = ctx.enter_context(tc.tile_pool(name="adjbuf", bufs=3))

        ident_f = consts.tile([P, P], dt.float32, tag="idf")
        make_identity(nc, ident_f)
        ident_b = consts.tile([P, P], dt.bfloat16, tag="idb")
        make_identity(nc, ident_b)
        ones_f = consts.tile([1, P], dt.float32, tag="onf")
        nc.vector.memset(ones_f, 1.0)
        ones_b = consts.tile([1, P], dt.bfloat16, tag="onb")
        nc.vector.memset(ones_b, 1.0)
        wext_sb = consts.tile([P, FO + 1], dt.float32, tag="wext")
        nc.sync.dma_start(out=wext_sb, in_=wext[:, :])
        wa2_sb = consts.tile([P, 1], dt.float32, tag="wa2")
        nc.sync.dma_start(out=wa2_sb, in_=wa2[:, :])
        bias_mh = consts.tile([P, 1], dt.float32, tag="bmh")
        nc.vector.memset(bias_mh, -M_SHIFT / 2)
        bias_z = consts.tile([P, 1], dt.float32, tag="bz")
        nc.vector.memset(bias_z, 0.0)

        for g in range(bpc):
            # ---------------- per-graph precompute (small) ----------------
            atoms_sb = gbuf.tile([P, NCH, F_IN], dt.float32, tag="atoms")
            nc.sync.dma_start(out=atoms_sb, in_=atoms[g].rearrange("(c p) f -> p c f", p=P))

            # transpose atoms chunks: atT[:, c, :] = [feat, node]
            atT_sb = gbuf.tile([P, NCH, P], dt.float32, tag="atT")
            for h in range(2):
                atT_ps = psum.tile([P, 4, P], dt.float32, tag="ps", name=f"atT_ps_{g}_{h}")
                for cc in range(4):
                    c = h * 4 + cc
                    nc.tensor.transpose(atT_ps[:, cc, :], atoms_sb[:, c, :], ident_f)
                nc.vector.tensor_copy(out=atT_sb[:, h * 4:(h + 1) * 4, :], in_=atT_ps)

            # Wh (bf16, with ones column) and s1 columns
            whones = gbuf.tile([P, NCH, FO + 1], dt.bfloat16, tag="whones")
            nc.vector.memset(whones[:, :, FO:FO + 1], 1.0)
            s1cols = gbuf.tile([P, NCH], dt.float32, tag="s1cols")
            for h in range(2):
                whc_ps = psum.tile([P, 4, FO + 1], dt.float32, tag="ps", name=f"whc_ps_{g}_{h}")
                for cc in range(4):
                    c = h * 4 + cc
                    nc.tensor.matmul(whc_ps[:, cc, :], lhsT=atT_sb[:, c, :], rhs=wext_sb,
                                     start=True, stop=True)
                nc.vector.tensor_copy(out=whones[:, h * 4:(h + 1) * 4, 0:FO],
                                      in_=whc_ps[:, :, 0:FO])
                nc.vector.tensor_copy(out=s1cols[:, h * 4:(h + 1) * 4],
                                      in_=whc_ps[:, :, FO])

            # s2 as a row: s2row[0, c*128+n] = sum_f atT[f, c, n] * wa2[f]
            s2row_sb = gbuf.tile([1, N], dt.float32, tag="s2row")
            for h in range(2):
                s2row_ps = psum.tile([1, 512], dt.float32, tag="ps", name=f"s2row_ps_{g}_{h}")
                for cc in range(4):
                    c = h * 4 + cc
                    nc.tensor.matmul(s2row_ps[:, cc * P:(cc + 1) * P], lhsT=wa2_sb,
                                     rhs=atT_sb[:, c, :], start=True, stop=True)
                nc.scalar.copy(out=s2row_sb[:, h * 512:(h + 1) * 512], in_=s2row_ps)

            # q row: exp(0.1*s2 - M/2), bf16
            qrow_sb = gbuf.tile([1, N], dt.bfloat16, tag="qrow")
            nc.scalar.activation(qrow_sb, s2row_sb, Act.Exp, bias=bias_mh[0:1, :], scale=0.1)

            # broadcast s2 (f32) and q (bf16) across partitions via K=1 matmul
            s2b_sb = gbuf.tile([P, N], dt.float32, tag="s2b")
            qb_sb = gbuf.tile([P, N], dt.bfloat16, tag="qb")
            for h in range(2):
                s2b_ps = psum.tile([P, 512], dt.float32, tag="ps", name=f"s2b_ps_{g}_{h}")
                nc.tensor.matmul(s2b_ps, lhsT=ones_f, rhs=s2row_sb[:, h * 512:(h + 1) * 512],
                                 start=True, stop=True)
                nc.scalar.copy(out=s2b_sb[:, h * 512:(h + 1) * 512], in_=s2b_ps)
                qb_ps = psum.tile([P, 512], dt.float32, tag="ps", name=f"qb_ps_{g}_{h}")
                nc.tensor.matmul(qb_ps, lhsT=ones_b, rhs=qrow_sb[:, h * 512:(h + 1) * 512],
                                 start=True, stop=True)
                nc.scalar.copy(out=qb_sb[:, h * 512:(h + 1) * 512], in_=qb_ps)

            # bias1[i] = s1_i - M ; p[i] = exp(0.1*s1_i - M/2)
            b1cols = gbuf.tile([P, NCH], dt.float32, tag="b1cols")
            nc.vector.tensor_scalar(b1cols, s1cols, -M_SHIFT, None, Alu.add)
            pcols = gbuf.tile([P, NCH], dt.float32, tag="pcols")
            nc.scalar.activation(pcols, s1cols, Act.Exp, bias=bias_mh, scale=0.1)

            res_g = gbuf.tile([P, NCH, FO], dt.float32, tag="res")

            # ---------------- main loop over i-chunks ----------------
            for ic in range(NCH):
                adj_sb = adjbuf.tile([P, N], dt.int32, tag="adj", name=f"adj_{g}_{ic}")
                nc.sync.dma_start(out=adj_sb, in_=adj[g, ic * P:(ic + 1) * P, :])
                adj_bf = cbuf.tile([P, N], dt.bfloat16, tag="adjbf", name=f"adjbf_{g}_{ic}")
                nc.vector.tensor_copy(out=adj_bf, in_=adj_sb)

                # t1 = exp(s2_j + s1_i - M)
                t1 = cbuf.tile([P, N], dt.bfloat16, tag="t1", name=f"t1_{g}_{ic}")
                nc.scalar.activation(t1, s2b_sb, Act.Exp, bias=b1cols[:, ic:ic + 1], scale=1.0)
                # t2 = exp(0.1*(s2_j + s1_i) - M)
                t2 = cbuf.tile([P, N], dt.bfloat16, tag="t2", name=f"t2_{g}_{ic}")
                nc.vector.tensor_scalar(t2, qb_sb, pcols[:, ic:ic + 1], None, Alu.mult)
                # E = exp(leaky_relu(s) - M);  P = E * adj
                em = cbuf.tile([P, N], dt.bfloat16, tag="em", name=f"em_{g}_{ic}")
                nc.vector.tensor_tensor(em, t1, t2, Alu.max)
                pm = cbuf.tile([P, N], dt.bfloat16, tag="pm", name=f"pm_{g}_{ic}")
                nc.vector.tensor_tensor(pm, em, adj_bf, Alu.mult)

                # transpose P blocks: pt[:, jc, :] = P[:, jc]^T  ([j, i])
                pt_ps = psum.tile([P, NCH, P], dt.bfloat16, tag="ps", name=f"pt_ps_{g}_{ic}")
                for jc in range(NCH):
                    nc.tensor.transpose(pt_ps[:, jc, :], pm[:, jc * P:(jc + 1) * P], ident_b)
                pt_sb = cbuf.tile([P, NCH, P], dt.bfloat16, tag="pt", name=f"pt_{g}_{ic}")
                if ic % 2 == 0:
                    nc.scalar.copy(out=pt_sb, in_=pt_ps)
                else:
                    nc.vector.tensor_copy(out=pt_sb, in_=pt_ps)

                # h'[i, 0:64] + denom col: sum_j P^T[j,i] * [Wh|1][j,:]
                h_ps = psum.tile([P, FO + 1], dt.float32, tag="ps", name=f"h_ps_{g}_{ic}")
                for jc in range(NCH):
                    nc.tensor.matmul(h_ps, lhsT=pt_sb[:, jc, :], rhs=whones[:, jc, :],
                                     start=(jc == 0), stop=(jc == NCH - 1))

                # finalize: divide by denom, ELU
                rec = cbuf.tile([P, 1], dt.float32, tag="rec", name=f"rec_{g}_{ic}")
                nc.vector.reciprocal(rec, h_ps[:, FO:FO + 1])
                hdiv = cbuf.tile([P, FO], dt.float32, tag="hdiv", name=f"hdiv_{g}_{ic}")
                nc.vector.tensor_scalar(hdiv, h_ps[:, 0:FO], rec, None, Alu.mult)
                hexp = cbuf.tile([P, FO], dt.float32, tag="hexp", name=f"hexp_{g}_{ic}")
                nc.scalar.activation(hexp, hdiv, Act.Exp, bias=bias_z)
                em1 = cbuf.tile([P, FO], dt.float32, tag="em1", name=f"em1_{g}_{ic}")
                nc.vector.tensor_scalar(em1, hexp, -1.0, 0.0, Alu.add, Alu.min)
                nc.vector.tensor_tensor(res_g[:, ic, :], hdiv, em1, Alu.max)

            nc.sync.dma_start(out=out[g].rearrange("(c p) f -> p c f", p=P), in_=res_g)

    # HW allows at most one sync-wait per Matmult/Ldweights; Tile can emit
    # more.  Run the bacc lowering passes that move extra waits onto
    # ldweights / standalone event-semaphore instructions.
    import bass_rust as _br
    _br.move_matmul_waits_to_ldweights(nc.m)
    _br.generate_event_semaphores(nc)
    return nc


_NC_CACHE: dict[int, bass.Bass] = {}


def _get_nc(bpc: int) -> bass.Bass:
    if bpc not in _NC_CACHE:
        _NC_CACHE[bpc] = build_gat(bpc)
    return _NC_CACHE[bpc]


def kernel(atoms_vector: np.ndarray, adjacency: np.ndarray, W: np.ndarray,
           a: np.ndarray) -> np.ndarray:
    from concourse.bass_utils import run_bass_kernel_spmd

    B = atoms_vector.shape[0]
    bpc = B // N_CORES
    a1 = a[:FO, :]
    a2 = a[FO:, :]
    wext = np.concatenate([W, W @ a1], axis=1).astype(np.float32)
    wa2 = (W @ a2).astype(np.float32)

    nc = _get_nc(bpc)
    in_maps = []
    for i in range(N_CORES):
        sl = slice(i * bpc, (i + 1) * bpc)
        in_maps.append({
            "atoms": np.ascontiguousarray(atoms_vector[sl]).astype(np.float32, copy=False),
            "adj": np.ascontiguousarray(adjacency[sl]).astype(np.int32, copy=False),
            "wext": wext,
            "wa2": wa2,
        })
    res = run_bass_kernel_spmd(nc, in_maps, list(range(N_CORES)))
    return np.concatenate([res.results[i]["out"] for i in range(N_CORES)], axis=0)
